# revision 7
# baseline (speedup 1.0000x reference)
"""Trainium2 Bass kernel for BroadcastResidualBlock.

Reference computation (per image, NHWC, H=W=19, C=256, HW=361):
    h1 = relu(bn1(x @ conv1_w + conv1_b))          # 1x1 conv = channel mix
    h2 = relu(dense(h1 over flattened board))       # spatial mix, per channel
    h3 = relu(bn2(h2 @ conv2_w + conv2_b))          # 1x1 conv
    out = x + h3

Strategy: pure data parallel over batch N=256 -> 32 images per core on 8
cores.  BN (inference) folds into the conv weights/biases on the host.  The
host also pre-transposes x into "C-layout" (N, C, HW) so every device-side
matmul contracts over the partition dimension with zero on-device transposes:

    s1: psum[r,  d] += xC_bf16[c_chunk, r_chunk].T @ w1[c_chunk, d]   (h1: S-layout)
    s2: psum[c,  q] += h1[p_chunk, c_chunk].T     @ dw[p_chunk, q]    (h2: C-layout)
    s3: psum[d,  q] += w2[c_chunk, d_chunk].T     @ h2[c_chunk, q]    (h3: C-layout)
    out = relu(psum3) + xC   (single fused VectorE op), stored in C-layout.

Matmuls run in bf16 (fp32 PSUM accumulation); x stays fp32 for the residual.
The host transposes the output back to NHWC.

Per-core steady state (cost-model): PE ~72us, DMA ~67us, ACT ~49us, DVE ~41us.
DMAs are batched 4 images per transfer and all weights ship as one blob so the
HWDGE/sequencer fixed costs (~625ns+900ns per DMA) stay off the critical path.
Each stage's PSUM lives in one 2-bank [128, 1024] tile so the whole epilogue
of a stage is a single DVE/ACT instruction.
"""

import numpy as np
import ml_dtypes

import concourse.bass as bass
import concourse.mybir as mybir
import concourse.tile as tile
from concourse import bacc
from concourse.bass_utils import run_bass_kernel_spmd

N_CORES = 8
NIMG = 32            # images per core
B = 4                # images per DMA batch
C = 256
HW = 361             # 19*19
P = 128
EPS = 1e-3
W_COLS = 2 * C + 3 * HW + 2 * C  # weight blob free size: w1 | dw | w2

F32 = mybir.dt.float32
BF16 = mybir.dt.bfloat16
AF = mybir.ActivationFunctionType
ALU = mybir.AluOpType

_prog_cache = {}


def build_program(has_b1: bool, has_b2: bool, has_b3: bool, reps: int = 1):
    nc = bacc.Bacc("TRN2", target_bir_lowering=False, debug=False)

    xc = nc.dram_tensor("xc", [NIMG, 2, P, HW], F32, kind="ExternalInput").ap()
    wb = nc.dram_tensor("wb", [P, W_COLS], BF16, kind="ExternalInput").ap()
    b1 = b2 = b3 = None
    if has_b1:
        b1 = nc.dram_tensor("b1", [P, 3 * C], F32, kind="ExternalInput").ap()
    if has_b2:
        b2 = nc.dram_tensor("b2", [P, 2, HW], F32, kind="ExternalInput").ap()
    if has_b3:
        b3 = nc.dram_tensor("b3", [2, P], F32, kind="ExternalInput").ap()
    yc = nc.dram_tensor("yc", [NIMG, 2, P, HW], F32, kind="ExternalOutput").ap()

    with tile.TileContext(nc) as tc:
        with (
            tc.tile_pool(name="const", bufs=1) as cpool,
            tc.tile_pool(name="xf", bufs=4) as xf_pool,
            tc.tile_pool(name="xb", bufs=4) as xb_pool,
            tc.tile_pool(name="h1", bufs=3) as h1_pool,
            tc.tile_pool(name="h2", bufs=3) as h2_pool,
            tc.tile_pool(name="yo", bufs=2) as yo_pool,
            tc.tile_pool(name="ps", bufs=4, space="PSUM") as ps_pool,
        ):
            wsb = cpool.tile([P, W_COLS], BF16)
            nc.sync.dma_start(wsb[:], wb)
            # views into the weight blob
            O_DW = 2 * C
            O_W2 = 2 * C + 3 * HW

            def w1_ap(cc):                      # [128, 256] rhs for s1
                return wsb[:, cc * C : (cc + 1) * C]

            def dw_ap(pc, k):                   # [k, 361] rhs for s2
                return wsb[:k, O_DW + pc * HW : O_DW + (pc + 1) * HW]

            def w2_ap(cc, dc):                  # [128, 128] lhsT for s3
                o = O_W2 + cc * C + dc * P
                return wsb[:, o : o + P]

            b1sb = b2sb = b3sb = None
            if has_b1:
                b1sb = cpool.tile([P, 3 * C], F32)
                nc.sync.dma_start(b1sb[:], b1)
            if has_b2:
                b2sb = cpool.tile([P, 2, HW], F32)
                nc.sync.dma_start(b2sb[:], b2)
            if has_b3:
                b3sb = cpool.tile([P, 2], F32)
                nc.sync.dma_start(b3sb[:], b3.rearrange("co ci -> ci co"))

            def emit_load(bi):
                xf = xf_pool.tile([P, B, 2, HW], F32, tag="xf")
                nc.sync.dma_start(
                    xf[:], xc[bi * B : (bi + 1) * B].rearrange("n co ci q -> ci n co q"))
                xb = xb_pool.tile([P, B, 2, HW], BF16, tag="xb")
                nc.vector.tensor_copy(xb[:], xf[:])
                return xf, xb

            def emit_s1(i, xb):
                k = i % B
                h1 = h1_pool.tile([P, 3, C], BF16, tag="h1")
                ps = ps_pool.tile([P, 1024], F32, tag="ps")
                for rc in range(3):
                    m = 128 if rc < 2 else 105
                    for cc in range(2):
                        nc.tensor.matmul(
                            ps[:m, rc * C : rc * C + C],
                            xb[:, k, cc, rc * 128 : rc * 128 + m],
                            w1_ap(cc),
                            start=(cc == 0),
                            stop=(cc == 1),
                        )
                if b1sb is not None:
                    nc.vector.scalar_tensor_tensor(
                        ps[:, : 3 * C], ps[:, : 3 * C], 0.0, b1sb[:],
                        ALU.bypass, ALU.add)
                nc.scalar.activation(
                    h1[:].rearrange("p a b -> p (a b)"), ps[:, : 3 * C], AF.Relu)
                return h1

            def emit_s2(i, h1):
                h2 = h2_pool.tile([P, 2, HW], BF16, tag="h2")
                ps = ps_pool.tile([P, 1024], F32, tag="ps")
                for cc in range(2):
                    for pc in range(3):
                        k = 128 if pc < 2 else 105
                        nc.tensor.matmul(
                            ps[:, cc * 512 : cc * 512 + HW],
                            h1[:k, pc, cc * 128 : (cc + 1) * 128],
                            dw_ap(pc, k),
                            start=(pc == 0),
                            stop=(pc == 2),
                        )
                psv = ps.rearrange("p (c x) -> p c x", c=2)[:, :, :HW]
                if b2sb is not None:
                    nc.vector.scalar_tensor_tensor(
                        psv, psv, 0.0, b2sb[:], ALU.bypass, ALU.add)
                nc.scalar.activation(h2[:], psv, AF.Relu)
                return h2

            def emit_s3(i, xf, yo, h2):
                k = i % B
                ps = ps_pool.tile([P, 1024], F32, tag="ps")
                for dc in range(2):
                    for cc in range(2):
                        nc.tensor.matmul(
                            ps[:, dc * 512 : dc * 512 + HW],
                            w2_ap(cc, dc),
                            h2[:, cc, :],
                            start=(cc == 0),
                            stop=(cc == 1),
                        )
                psv = ps.rearrange("p (c x) -> p c x", c=2)[:, :, :HW]
                if b3sb is not None:
                    for dc in range(2):
                        nc.scalar.activation(
                            yo[:, k, dc, :], psv[:, dc, :], AF.Relu,
                            bias=b3sb[:, dc : dc + 1])
                    nc.vector.tensor_add(
                        yo[:, k, :, :], yo[:, k, :, :], xf[:, k, :, :])
                else:
                    nc.vector.scalar_tensor_tensor(
                        yo[:, k, :, :], psv, 0.0, xf[:, k, :, :],
                        ALU.max, ALU.add)

            def emit_store(bi, yo):
                # SWDGE path: keeps store DMAs (which wait on compute) off the
                # sync queue so they never head-of-line-block prefetch loads
                nc.gpsimd.dma_start(
                    yc[bi * B : (bi + 1) * B].rearrange("n co ci q -> ci n co q"),
                    yo[:])

            def body():
                # software pipeline: s1(i) | s2(i-1) | s3(i-2); batch loads
                # prefetched two steps ahead, stores flushed per batch
                xfs, xbs, h1s, h2s, yos = {}, {}, {}, {}, {}
                xfs[0], xbs[0] = emit_load(0)
                xfs[1], xbs[1] = emit_load(1)
                for step in range(NIMG + 2):
                    nb = (step + 6) // B
                    if (step + 6) % B == 0 and nb < NIMG // B:
                        xfs[nb], xbs[nb] = emit_load(nb)
                    if step >= 2:
                        i = step - 2
                        bi = i // B
                        if i % B == 0:
                            yos[bi] = yo_pool.tile(
                                [P, B, 2, HW], F32, tag="yo", name="yo")
                        emit_s3(i, xfs[bi], yos[bi], h2s.pop(i))
                        if i % B == B - 1:
                            emit_store(bi, yos.pop(bi))
                            xfs.pop(bi)
                    if 1 <= step <= NIMG:
                        h2s[step - 1] = emit_s2(step - 1, h1s.pop(step - 1))
                    if step < NIMG:
                        i = step
                        h1s[i] = emit_s1(i, xbs[i // B])
                        if i % B == B - 1:
                            xbs.pop(i // B)

            if reps == 1:
                body()
            else:
                with tc.For_i(0, reps, 1):
                    body()

    nc.compile()
    return nc


def _get_program(key):
    if key not in _prog_cache:
        _prog_cache[key] = build_program(*key)
    return _prog_cache[key]


def _marshal(x, conv1_w, conv1_b, bn1_mean, bn1_var, bn1_beta,
             dense_w, dense_b, conv2_w, conv2_b, bn2_mean, bn2_var, bn2_beta):
    bf16 = ml_dtypes.bfloat16
    n = x.shape[0]
    rs1 = 1.0 / np.sqrt(bn1_var.astype(np.float64) + EPS)
    rs2 = 1.0 / np.sqrt(bn2_var.astype(np.float64) + EPS)
    w1f = conv1_w.astype(np.float64) * rs1[None, :]
    w2f = conv2_w.astype(np.float64) * rs2[None, :]
    b1f = (conv1_b - bn1_mean).astype(np.float64) * rs1 + bn1_beta
    b2f = dense_b.astype(np.float64)
    b3f = (conv2_b - bn2_mean).astype(np.float64) * rs2 + bn2_beta
    has_b1 = bool(np.any(b1f != 0.0))
    has_b2 = bool(np.any(b2f != 0.0))
    has_b3 = bool(np.any(b3f != 0.0))

    # weight blob [128, W_COLS]: per partition ci the columns are
    #   w1[cc=0..1] (256 each) | dw[pc=0..2] (361 each) | w2[cc=0..1] (256 each)
    blob = np.zeros((P, W_COLS), np.float64)
    w1r = w1f.reshape(2, P, C)
    for cc in range(2):
        blob[:, cc * C : (cc + 1) * C] = w1r[cc]
    dwp = np.zeros((3 * P, HW), np.float64)
    dwp[:HW] = dense_w
    dwr = dwp.reshape(3, P, HW)
    for pc in range(3):
        blob[:, 2 * C + pc * HW : 2 * C + (pc + 1) * HW] = dwr[pc]
    w2r = w2f.reshape(2, P, C)
    for cc in range(2):
        blob[:, 2 * C + 3 * HW + cc * C : 2 * C + 3 * HW + (cc + 1) * C] = w2r[cc]
    wbb = blob.astype(bf16)

    x_c = np.ascontiguousarray(
        x.reshape(n, HW, C).transpose(0, 2, 1)
    ).reshape(N_CORES, NIMG, 2, P, HW)

    in_maps = []
    for c in range(N_CORES):
        m = {"xc": x_c[c], "wb": wbb}
        if has_b1:
            m["b1"] = np.ascontiguousarray(np.broadcast_to(
                np.tile(b1f, 3).astype(np.float32), (P, 3 * C)))
        if has_b2:
            m["b2"] = np.ascontiguousarray(np.broadcast_to(
                b2f.astype(np.float32), (P, 2, HW)))
        if has_b3:
            m["b3"] = np.ascontiguousarray(
                b3f.astype(np.float32).reshape(2, P))
        in_maps.append(m)
    return (has_b1, has_b2, has_b3), in_maps


def _unmarshal(results, n, h, w):
    y = np.stack([results[c]["yc"] for c in range(N_CORES)])
    y = y.reshape(n, C, HW).transpose(0, 2, 1)
    return np.ascontiguousarray(y.reshape(n, h, w, C).astype(np.float32))


def kernel(x, conv1_w, conv1_b, bn1_mean, bn1_var, bn1_beta,
           dense_w, dense_b, conv2_w, conv2_b, bn2_mean, bn2_var, bn2_beta):
    n, h, w, _ = x.shape
    flags, in_maps = _marshal(
        x, conv1_w, conv1_b, bn1_mean, bn1_var, bn1_beta,
        dense_w, dense_b, conv2_w, conv2_b, bn2_mean, bn2_var, bn2_beta)
    nc = _get_program((*flags, 1))
    res = run_bass_kernel_spmd(nc, in_maps, list(range(N_CORES)))
    return _unmarshal(res.results, n, h, w)


# revision 8
# speedup vs baseline: 1.1337x; 1.1337x over previous
"""Trainium2 Bass kernel for BroadcastResidualBlock.

Reference computation (per image, NHWC, H=W=19, C=256, HW=361):
    h1 = relu(bn1(x @ conv1_w + conv1_b))          # 1x1 conv = channel mix
    h2 = relu(dense(h1 over flattened board))       # spatial mix, per channel
    h3 = relu(bn2(h2 @ conv2_w + conv2_b))          # 1x1 conv
    out = x + h3

Strategy: pure data parallel over batch N=256 -> 32 images per core on 8
cores.  BN (inference) folds into the conv weights/biases on the host.  The
host also pre-transposes x into "C-layout" (N, C, HW) so every device-side
matmul contracts over the partition dimension with zero on-device transposes:

    s1: psum[r,  d] += xC_bf16[c_chunk, r_chunk].T @ w1[c_chunk, d]   (h1: S-layout)
    s2: psum[c,  q] += h1[p_chunk, c_chunk].T     @ dw[p_chunk, q]    (h2: C-layout)
    s3: psum[d,  q] += w2[c_chunk, d_chunk].T     @ h2[c_chunk, q]    (h3: C-layout)
    out = relu(psum3) + xC   (single fused VectorE op), stored in C-layout.

Matmuls run in bf16 (fp32 PSUM accumulation); x stays fp32 for the residual.
The host transposes the output back to NHWC.

Per-core steady state (cost-model): PE ~72us, DMA ~67us, ACT ~49us, DVE ~41us.
DMAs are batched 4 images per transfer and all weights ship as one blob so the
HWDGE/sequencer fixed costs (~625ns+900ns per DMA) stay off the critical path.
Each stage's PSUM lives in one 2-bank [128, 1024] tile so the whole epilogue
of a stage is a single DVE/ACT instruction.
"""

import numpy as np
import ml_dtypes

import concourse.bass as bass
import concourse.mybir as mybir
import concourse.tile as tile
from concourse import bacc
from concourse.bass_utils import run_bass_kernel_spmd

N_CORES = 8
NIMG = 32            # images per core
B = 2                # images per DMA batch
C = 256
HW = 361             # 19*19
P = 128
EPS = 1e-3
W_COLS = 2 * C + 3 * HW + 2 * C  # weight blob free size: w1 | dw | w2

F32 = mybir.dt.float32
BF16 = mybir.dt.bfloat16
AF = mybir.ActivationFunctionType
ALU = mybir.AluOpType

_prog_cache = {}


def build_program(has_b1: bool, has_b2: bool, has_b3: bool, reps: int = 1):
    nc = bacc.Bacc("TRN2", target_bir_lowering=False, debug=False)

    xc = nc.dram_tensor("xc", [NIMG, 2, P, HW], F32, kind="ExternalInput").ap()
    wb = nc.dram_tensor("wb", [P, W_COLS], BF16, kind="ExternalInput").ap()
    b1 = b2 = b3 = None
    if has_b1:
        b1 = nc.dram_tensor("b1", [P, 3 * C], F32, kind="ExternalInput").ap()
    if has_b2:
        b2 = nc.dram_tensor("b2", [P, 2, HW], F32, kind="ExternalInput").ap()
    if has_b3:
        b3 = nc.dram_tensor("b3", [2, P], F32, kind="ExternalInput").ap()
    yc = nc.dram_tensor("yc", [NIMG, 2, P, HW], F32, kind="ExternalOutput").ap()

    with tile.TileContext(nc) as tc:
        with (
            tc.tile_pool(name="const", bufs=1) as cpool,
            tc.tile_pool(name="xf", bufs=5) as xf_pool,
            tc.tile_pool(name="xb", bufs=4) as xb_pool,
            tc.tile_pool(name="h1", bufs=3) as h1_pool,
            tc.tile_pool(name="h2", bufs=3) as h2_pool,
            tc.tile_pool(name="yo", bufs=3) as yo_pool,
            tc.tile_pool(name="ps", bufs=4, space="PSUM") as ps_pool,
        ):
            wsb = cpool.tile([P, W_COLS], BF16)
            # scalar queue: runs in parallel with the first x load on sync
            nc.scalar.dma_start(wsb[:], wb)
            # views into the weight blob
            O_DW = 2 * C
            O_W2 = 2 * C + 3 * HW

            def w1_ap(cc):                      # [128, 256] rhs for s1
                return wsb[:, cc * C : (cc + 1) * C]

            def dw_ap(pc, k):                   # [k, 361] rhs for s2
                return wsb[:k, O_DW + pc * HW : O_DW + (pc + 1) * HW]

            def w2_ap(cc, dc):                  # [128, 128] lhsT for s3
                o = O_W2 + cc * C + dc * P
                return wsb[:, o : o + P]

            b1sb = b2sb = b3sb = None
            if has_b1:
                b1sb = cpool.tile([P, 3 * C], F32)
                nc.sync.dma_start(b1sb[:], b1)
            if has_b2:
                b2sb = cpool.tile([P, 2, HW], F32)
                nc.sync.dma_start(b2sb[:], b2)
            if has_b3:
                b3sb = cpool.tile([P, 2], F32)
                nc.sync.dma_start(b3sb[:], b3.rearrange("co ci -> ci co"))

            def emit_load(bi):
                xf = xf_pool.tile([P, B, 2, HW], F32, tag="xf")
                nc.sync.dma_start(
                    xf[:], xc[bi * B : (bi + 1) * B].rearrange("n co ci q -> ci n co q"))
                xb = xb_pool.tile([P, B, 2, HW], BF16, tag="xb")
                nc.vector.tensor_copy(xb[:], xf[:])
                return xf, xb

            def emit_s1(i, xb):
                k = i % B
                h1 = h1_pool.tile([P, 3, C], BF16, tag="h1")
                ps = ps_pool.tile([P, 1024], F32, tag="ps")
                for rc in range(3):
                    m = 128 if rc < 2 else 105
                    for cc in range(2):
                        nc.tensor.matmul(
                            ps[:m, rc * C : rc * C + C],
                            xb[:, k, cc, rc * 128 : rc * 128 + m],
                            w1_ap(cc),
                            start=(cc == 0),
                            stop=(cc == 1),
                        )
                if b1sb is not None:
                    nc.vector.scalar_tensor_tensor(
                        ps[:, : 3 * C], ps[:, : 3 * C], 0.0, b1sb[:],
                        ALU.bypass, ALU.add)
                nc.scalar.activation(
                    h1[:].rearrange("p a b -> p (a b)"), ps[:, : 3 * C], AF.Relu)
                return h1

            def emit_s2(i, h1):
                h2 = h2_pool.tile([P, 2, HW], BF16, tag="h2")
                ps = ps_pool.tile([P, 1024], F32, tag="ps")
                for cc in range(2):
                    for pc in range(3):
                        k = 128 if pc < 2 else 105
                        nc.tensor.matmul(
                            ps[:, cc * 512 : cc * 512 + HW],
                            h1[:k, pc, cc * 128 : (cc + 1) * 128],
                            dw_ap(pc, k),
                            start=(pc == 0),
                            stop=(pc == 2),
                        )
                psv = ps.rearrange("p (c x) -> p c x", c=2)[:, :, :HW]
                if b2sb is not None:
                    nc.vector.scalar_tensor_tensor(
                        psv, psv, 0.0, b2sb[:], ALU.bypass, ALU.add)
                nc.scalar.activation(h2[:], psv, AF.Relu)
                return h2

            def emit_s3(i, xf, yo, h2):
                k = i % B
                ps = ps_pool.tile([P, 1024], F32, tag="ps")
                for dc in range(2):
                    for cc in range(2):
                        nc.tensor.matmul(
                            ps[:, dc * 512 : dc * 512 + HW],
                            w2_ap(cc, dc),
                            h2[:, cc, :],
                            start=(cc == 0),
                            stop=(cc == 1),
                        )
                psv = ps.rearrange("p (c x) -> p c x", c=2)[:, :, :HW]
                if b3sb is not None:
                    for dc in range(2):
                        nc.scalar.activation(
                            yo[:, k, dc, :], psv[:, dc, :], AF.Relu,
                            bias=b3sb[:, dc : dc + 1])
                    nc.vector.tensor_add(
                        yo[:, k, :, :], yo[:, k, :, :], xf[:, k, :, :])
                else:
                    nc.vector.scalar_tensor_tensor(
                        yo[:, k, :, :], psv, 0.0, xf[:, k, :, :],
                        ALU.max, ALU.add)

            def emit_store(bi, yo):
                # SWDGE path: keeps store DMAs (which wait on compute) off the
                # sync queue so they never head-of-line-block prefetch loads
                nc.gpsimd.dma_start(
                    yc[bi * B : (bi + 1) * B].rearrange("n co ci q -> ci n co q"),
                    yo[:])

            def body():
                # software pipeline: s1(i) | s2(i-1) | s3(i-2); batch loads
                # prefetched two steps ahead, stores flushed per batch
                xfs, xbs, h1s, h2s, yos = {}, {}, {}, {}, {}
                for pb in range(3):
                    xfs[pb], xbs[pb] = emit_load(pb)
                for step in range(NIMG + 2):
                    nb = (step + 6) // B
                    if (step + 6) % B == 0 and nb < NIMG // B:
                        xfs[nb], xbs[nb] = emit_load(nb)
                    # step order [s1(i), s3(i-2), s2(i-1)]: the s1 matmuls at
                    # the head of the step give ACT time to release the PSUM
                    # slot that s3's allocation reuses
                    if step < NIMG:
                        i = step
                        h1s[i] = emit_s1(i, xbs[i // B])
                        if i % B == B - 1:
                            xbs.pop(i // B)
                    if step >= 2:
                        i = step - 2
                        bi = i // B
                        if i % B == 0:
                            yos[bi] = yo_pool.tile(
                                [P, B, 2, HW], F32, tag="yo", name="yo")
                        emit_s3(i, xfs[bi], yos[bi], h2s.pop(i))
                        if i % B == B - 1:
                            emit_store(bi, yos.pop(bi))
                            xfs.pop(bi)
                    if 1 <= step <= NIMG:
                        h2s[step - 1] = emit_s2(step - 1, h1s.pop(step - 1))

            if reps == 1:
                body()
            else:
                with tc.For_i(0, reps, 1):
                    body()

    nc.compile()
    return nc


def _get_program(key):
    if key not in _prog_cache:
        _prog_cache[key] = build_program(*key)
    return _prog_cache[key]


def _marshal(x, conv1_w, conv1_b, bn1_mean, bn1_var, bn1_beta,
             dense_w, dense_b, conv2_w, conv2_b, bn2_mean, bn2_var, bn2_beta):
    bf16 = ml_dtypes.bfloat16
    n = x.shape[0]
    rs1 = 1.0 / np.sqrt(bn1_var.astype(np.float64) + EPS)
    rs2 = 1.0 / np.sqrt(bn2_var.astype(np.float64) + EPS)
    w1f = conv1_w.astype(np.float64) * rs1[None, :]
    w2f = conv2_w.astype(np.float64) * rs2[None, :]
    b1f = (conv1_b - bn1_mean).astype(np.float64) * rs1 + bn1_beta
    b2f = dense_b.astype(np.float64)
    b3f = (conv2_b - bn2_mean).astype(np.float64) * rs2 + bn2_beta
    has_b1 = bool(np.any(b1f != 0.0))
    has_b2 = bool(np.any(b2f != 0.0))
    has_b3 = bool(np.any(b3f != 0.0))

    # weight blob [128, W_COLS]: per partition ci the columns are
    #   w1[cc=0..1] (256 each) | dw[pc=0..2] (361 each) | w2[cc=0..1] (256 each)
    blob = np.zeros((P, W_COLS), np.float64)
    w1r = w1f.reshape(2, P, C)
    for cc in range(2):
        blob[:, cc * C : (cc + 1) * C] = w1r[cc]
    dwp = np.zeros((3 * P, HW), np.float64)
    dwp[:HW] = dense_w
    dwr = dwp.reshape(3, P, HW)
    for pc in range(3):
        blob[:, 2 * C + pc * HW : 2 * C + (pc + 1) * HW] = dwr[pc]
    w2r = w2f.reshape(2, P, C)
    for cc in range(2):
        blob[:, 2 * C + 3 * HW + cc * C : 2 * C + 3 * HW + (cc + 1) * C] = w2r[cc]
    wbb = blob.astype(bf16)

    x_c = np.ascontiguousarray(
        x.reshape(n, HW, C).transpose(0, 2, 1)
    ).reshape(N_CORES, NIMG, 2, P, HW)

    in_maps = []
    for c in range(N_CORES):
        m = {"xc": x_c[c], "wb": wbb}
        if has_b1:
            m["b1"] = np.ascontiguousarray(np.broadcast_to(
                np.tile(b1f, 3).astype(np.float32), (P, 3 * C)))
        if has_b2:
            m["b2"] = np.ascontiguousarray(np.broadcast_to(
                b2f.astype(np.float32), (P, 2, HW)))
        if has_b3:
            m["b3"] = np.ascontiguousarray(
                b3f.astype(np.float32).reshape(2, P))
        in_maps.append(m)
    return (has_b1, has_b2, has_b3), in_maps


def _unmarshal(results, n, h, w):
    y = np.stack([results[c]["yc"] for c in range(N_CORES)])
    y = y.reshape(n, C, HW).transpose(0, 2, 1)
    return np.ascontiguousarray(y.reshape(n, h, w, C).astype(np.float32))


def kernel(x, conv1_w, conv1_b, bn1_mean, bn1_var, bn1_beta,
           dense_w, dense_b, conv2_w, conv2_b, bn2_mean, bn2_var, bn2_beta):
    n, h, w, _ = x.shape
    flags, in_maps = _marshal(
        x, conv1_w, conv1_b, bn1_mean, bn1_var, bn1_beta,
        dense_w, dense_b, conv2_w, conv2_b, bn2_mean, bn2_var, bn2_beta)
    nc = _get_program((*flags, 1))
    res = run_bass_kernel_spmd(nc, in_maps, list(range(N_CORES)))
    return _unmarshal(res.results, n, h, w)


# revision 9
# speedup vs baseline: 1.3563x; 1.1963x over previous
"""Trainium2 Bass kernel for BroadcastResidualBlock.

Reference computation (per image, NHWC, H=W=19, C=256, HW=361):
    h1 = relu(bn1(x @ conv1_w + conv1_b))          # 1x1 conv = channel mix
    h2 = relu(dense(h1 over flattened board))       # spatial mix, per channel
    h3 = relu(bn2(h2 @ conv2_w + conv2_b))          # 1x1 conv
    out = x + h3

Strategy: pure data parallel over batch N=256 -> 32 images per core on 8
cores.  BN (inference) folds into the conv weights/biases on the host.  The
host also pre-transposes x into "C-layout" (N, C, HW) so every device-side
matmul contracts over the partition dimension with zero on-device transposes:

    s1: psum[r,  d] += xC_bf16[c_chunk, r_chunk].T @ w1[c_chunk, d]   (h1: S-layout)
    s2: psum[c,  q] += h1[p_chunk, c_chunk].T     @ dw[p_chunk, q]    (h2: C-layout)
    s3: psum[d,  q] += w2[c_chunk, d_chunk].T     @ h2[c_chunk, q]    (h3: C-layout)
    out = relu(psum3) + xC   (single fused VectorE op), stored in C-layout.

Matmuls run in bf16 (fp32 PSUM accumulation); x stays fp32 for the residual.
The host transposes the output back to NHWC.

Schedule: 3-stage software pipeline over images, one 1-bank PSUM tile per
matmul group (7 per image) so releases stagger through the step; epilogues are
split ACT/DVE to balance the engines; x loads ride the sync queue, output
stores the gpsimd (SWDGE) queue so stores never head-of-line-block prefetches.
"""

import numpy as np
import ml_dtypes

import concourse.bass as bass
import concourse.mybir as mybir
import concourse.tile as tile
from concourse import bacc
from concourse.bass_utils import run_bass_kernel_spmd

N_CORES = 8
NIMG = 32            # images per core
C = 256
HW = 361             # 19*19
P = 128
EPS = 1e-3
W_COLS = 2 * C + 3 * HW + 2 * C  # weight blob free size: w1 | dw | w2

F32 = mybir.dt.float32
BF16 = mybir.dt.bfloat16
AF = mybir.ActivationFunctionType
ALU = mybir.AluOpType

# DMA batches: singles at the edges (short critical path at startup/teardown),
# pairs in steady state
BATCHES = [[0], [1]] + [[i, i + 1] for i in range(2, 30, 2)] + [[30], [31]]
BMAX = 2

_prog_cache = {}


def build_program(has_b1: bool, has_b2: bool, has_b3: bool, reps: int = 1):
    nc = bacc.Bacc("TRN2", target_bir_lowering=False, debug=False)

    xc = nc.dram_tensor("xc", [NIMG, 2, P, HW], F32, kind="ExternalInput").ap()
    wb = nc.dram_tensor("wb", [P, W_COLS], BF16, kind="ExternalInput").ap()
    b1 = b2 = b3 = None
    if has_b1:
        b1 = nc.dram_tensor("b1", [P, 3 * C], F32, kind="ExternalInput").ap()
    if has_b2:
        b2 = nc.dram_tensor("b2", [P, 2, HW], F32, kind="ExternalInput").ap()
    if has_b3:
        b3 = nc.dram_tensor("b3", [2, P], F32, kind="ExternalInput").ap()
    yc = nc.dram_tensor("yc", [NIMG, 2, P, HW], F32, kind="ExternalOutput").ap()

    batch_of = {}
    for bi, imgs in enumerate(BATCHES):
        for k, i in enumerate(imgs):
            batch_of[i] = (bi, k)

    with tile.TileContext(nc) as tc:
        with (
            tc.tile_pool(name="const", bufs=1) as cpool,
            tc.tile_pool(name="xf", bufs=6) as xf_pool,
            tc.tile_pool(name="xb", bufs=5) as xb_pool,
            tc.tile_pool(name="h1", bufs=3) as h1_pool,
            tc.tile_pool(name="h2", bufs=3) as h2_pool,
            tc.tile_pool(name="yo", bufs=3) as yo_pool,
            tc.tile_pool(name="ps", bufs=8, space="PSUM") as ps_pool,
        ):
            wsb = cpool.tile([P, W_COLS], BF16)
            # scalar queue: runs in parallel with the first x load on sync
            nc.scalar.dma_start(wsb[:], wb)
            O_DW = 2 * C
            O_W2 = 2 * C + 3 * HW

            def w1_ap(cc):                      # [128, 256] rhs for s1
                return wsb[:, cc * C : (cc + 1) * C]

            def dw_ap(pc, k):                   # [k, 361] rhs for s2
                return wsb[:k, O_DW + pc * HW : O_DW + (pc + 1) * HW]

            def w2_ap(cc, dc):                  # [128, 128] lhsT for s3
                o = O_W2 + cc * C + dc * P
                return wsb[:, o : o + P]

            b1sb = b2sb = b3sb = None
            if has_b1:
                b1sb = cpool.tile([P, 3 * C], F32)
                nc.sync.dma_start(b1sb[:], b1)
            if has_b2:
                b2sb = cpool.tile([P, 2, HW], F32)
                nc.sync.dma_start(b2sb[:], b2)
            if has_b3:
                b3sb = cpool.tile([P, 2], F32)
                nc.sync.dma_start(b3sb[:], b3.rearrange("co ci -> ci co"))

            def emit_load(bi):
                imgs = BATCHES[bi]
                nb = len(imgs)
                xf = xf_pool.tile([P, BMAX, 2, HW], F32, tag="xf", name="xf")
                nc.sync.dma_start(
                    xf[:, :nb],
                    xc[imgs[0] : imgs[0] + nb].rearrange("n co ci q -> ci n co q"))
                xb = xb_pool.tile([P, BMAX, 2, HW], BF16, tag="xb", name="xb")
                nc.vector.tensor_copy(xb[:, :nb], xf[:, :nb])
                return xf, xb

            def emit_s1_group(i, xb, k, rc, h1):
                m = 128 if rc < 2 else 105
                ps = ps_pool.tile([P, 512], F32, tag="ps", name="ps")
                for cc in range(2):
                    nc.tensor.matmul(
                        ps[:m, :C],
                        xb[:, k, cc, rc * 128 : rc * 128 + m],
                        w1_ap(cc),
                        start=(cc == 0),
                        stop=(cc == 1),
                    )
                if b1sb is not None:
                    nc.vector.scalar_tensor_tensor(
                        ps[:m, :C], ps[:m, :C], 0.0,
                        b1sb[:m, rc * C : (rc + 1) * C], ALU.bypass, ALU.add)
                if rc < 2:
                    nc.scalar.activation(h1[:m, rc, :], ps[:m, :C], AF.Relu)
                else:
                    nc.vector.tensor_scalar_max(h1[:m, rc, :], ps[:m, :C], 0.0)

            def emit_s2_group(i, h1, cc, h2):
                ps = ps_pool.tile([P, 512], F32, tag="ps", name="ps")
                for pc in range(3):
                    k = 128 if pc < 2 else 105
                    nc.tensor.matmul(
                        ps[:, :HW],
                        h1[:k, pc, cc * 128 : (cc + 1) * 128],
                        dw_ap(pc, k),
                        start=(pc == 0),
                        stop=(pc == 2),
                    )
                if b2sb is not None:
                    nc.vector.scalar_tensor_tensor(
                        ps[:, :HW], ps[:, :HW], 0.0, b2sb[:, cc, :],
                        ALU.bypass, ALU.add)
                nc.scalar.activation(h2[:, cc, :], ps[:, :HW], AF.Relu)

            def emit_s3_group(i, xf, k, yo, h2, dc):
                ps = ps_pool.tile([P, 512], F32, tag="ps", name="ps")
                for cc in range(2):
                    nc.tensor.matmul(
                        ps[:, :HW],
                        w2_ap(cc, dc),
                        h2[:, cc, :],
                        start=(cc == 0),
                        stop=(cc == 1),
                    )
                if b3sb is not None:
                    nc.scalar.activation(
                        yo[:, k, dc, :], ps[:, :HW], AF.Relu,
                        bias=b3sb[:, dc : dc + 1])
                    nc.vector.tensor_add(
                        yo[:, k, dc, :], yo[:, k, dc, :], xf[:, k, dc, :])
                else:
                    nc.vector.scalar_tensor_tensor(
                        yo[:, k, dc, :], ps[:, :HW], 0.0, xf[:, k, dc, :],
                        ALU.max, ALU.add)

            def emit_store(bi, yo):
                imgs = BATCHES[bi]
                nb = len(imgs)
                # SWDGE path: keeps store DMAs (which wait on compute) off the
                # sync queue so they never head-of-line-block prefetch loads
                nc.gpsimd.dma_start(
                    yc[imgs[0] : imgs[0] + nb].rearrange("n co ci q -> ci n co q"),
                    yo[:, :nb])

            def body():
                # software pipeline: s1(i) | s2(i-1) | s3(i-2), interleaved at
                # matmul-group granularity so PSUM slot releases stagger
                xfs, xbs, h1s, h2s, yos = {}, {}, {}, {}, {}
                loaded = 0
                for pb in range(4):
                    xfs[pb], xbs[pb] = emit_load(pb)
                    loaded += 1
                for step in range(NIMG + 2):
                    if step % 2 == 0 and loaded < len(BATCHES):
                        xfs[loaded], xbs[loaded] = emit_load(loaded)
                        loaded += 1
                    i1 = step if step < NIMG else None
                    i2 = step - 1 if 1 <= step <= NIMG else None
                    i3 = step - 2 if step >= 2 else None
                    if i1 is not None:
                        h1s[i1] = h1_pool.tile([P, 3, C], BF16, tag="h1", name="h1")
                    bi3 = k3 = None
                    if i3 is not None:
                        bi3, k3 = batch_of[i3]
                        if k3 == 0:
                            yos[bi3] = yo_pool.tile(
                                [P, BMAX, 2, HW], F32, tag="yo", name="yo")
                    if i2 is not None:
                        h2s[i2] = h2_pool.tile([P, 2, HW], BF16, tag="h2", name="h2")
                    # interleave matmul groups: s1 x3 | s3 x2 | s2 x2
                    for g in range(3):
                        if i1 is not None:
                            b1i, k1 = batch_of[i1]
                            emit_s1_group(i1, xbs[b1i], k1, g, h1s[i1])
                        if g < 2 and i3 is not None:
                            emit_s3_group(i3, xfs[bi3], k3, yos[bi3], h2s[i3], g)
                        if g < 2 and i2 is not None:
                            emit_s2_group(i2, h1s[i2], g, h2s[i2])
                    if i1 is not None:
                        b1i, k1 = batch_of[i1]
                        if k1 == len(BATCHES[b1i]) - 1:
                            xbs.pop(b1i)
                    if i2 is not None:
                        h1s.pop(i2)
                    if i3 is not None:
                        h2s.pop(i3)
                        if k3 == len(BATCHES[bi3]) - 1:
                            emit_store(bi3, yos.pop(bi3))
                            xfs.pop(bi3)

            if reps == 1:
                body()
            else:
                with tc.For_i(0, reps, 1):
                    body()

    nc.compile()
    return nc


def _get_program(key):
    if key not in _prog_cache:
        _prog_cache[key] = build_program(*key)
    return _prog_cache[key]


def _marshal(x, conv1_w, conv1_b, bn1_mean, bn1_var, bn1_beta,
             dense_w, dense_b, conv2_w, conv2_b, bn2_mean, bn2_var, bn2_beta):
    bf16 = ml_dtypes.bfloat16
    n = x.shape[0]
    rs1 = 1.0 / np.sqrt(bn1_var.astype(np.float64) + EPS)
    rs2 = 1.0 / np.sqrt(bn2_var.astype(np.float64) + EPS)
    w1f = conv1_w.astype(np.float64) * rs1[None, :]
    w2f = conv2_w.astype(np.float64) * rs2[None, :]
    b1f = (conv1_b - bn1_mean).astype(np.float64) * rs1 + bn1_beta
    b2f = dense_b.astype(np.float64)
    b3f = (conv2_b - bn2_mean).astype(np.float64) * rs2 + bn2_beta
    has_b1 = bool(np.any(b1f != 0.0))
    has_b2 = bool(np.any(b2f != 0.0))
    has_b3 = bool(np.any(b3f != 0.0))

    # weight blob [128, W_COLS]: per partition ci the columns are
    #   w1[cc=0..1] (256 each) | dw[pc=0..2] (361 each) | w2[cc=0..1] (256 each)
    blob = np.zeros((P, W_COLS), np.float64)
    w1r = w1f.reshape(2, P, C)
    for cc in range(2):
        blob[:, cc * C : (cc + 1) * C] = w1r[cc]
    dwp = np.zeros((3 * P, HW), np.float64)
    dwp[:HW] = dense_w
    dwr = dwp.reshape(3, P, HW)
    for pc in range(3):
        blob[:, 2 * C + pc * HW : 2 * C + (pc + 1) * HW] = dwr[pc]
    w2r = w2f.reshape(2, P, C)
    for cc in range(2):
        blob[:, 2 * C + 3 * HW + cc * C : 2 * C + 3 * HW + (cc + 1) * C] = w2r[cc]
    wbb = blob.astype(bf16)

    x_c = np.ascontiguousarray(
        x.reshape(n, HW, C).transpose(0, 2, 1)
    ).reshape(N_CORES, NIMG, 2, P, HW)

    in_maps = []
    for c in range(N_CORES):
        m = {"xc": x_c[c], "wb": wbb}
        if has_b1:
            m["b1"] = np.ascontiguousarray(np.broadcast_to(
                np.tile(b1f, 3).astype(np.float32), (P, 3 * C)))
        if has_b2:
            m["b2"] = np.ascontiguousarray(np.broadcast_to(
                b2f.astype(np.float32), (P, 2, HW)))
        if has_b3:
            m["b3"] = np.ascontiguousarray(
                b3f.astype(np.float32).reshape(2, P))
        in_maps.append(m)
    return (has_b1, has_b2, has_b3), in_maps


def _unmarshal(results, n, h, w):
    y = np.stack([results[c]["yc"] for c in range(N_CORES)])
    y = y.reshape(n, C, HW).transpose(0, 2, 1)
    return np.ascontiguousarray(y.reshape(n, h, w, C).astype(np.float32))


def kernel(x, conv1_w, conv1_b, bn1_mean, bn1_var, bn1_beta,
           dense_w, dense_b, conv2_w, conv2_b, bn2_mean, bn2_var, bn2_beta):
    n, h, w, _ = x.shape
    flags, in_maps = _marshal(
        x, conv1_w, conv1_b, bn1_mean, bn1_var, bn1_beta,
        dense_w, dense_b, conv2_w, conv2_b, bn2_mean, bn2_var, bn2_beta)
    nc = _get_program((*flags, 1))
    res = run_bass_kernel_spmd(nc, in_maps, list(range(N_CORES)))
    return _unmarshal(res.results, n, h, w)


# revision 17
# speedup vs baseline: 1.3903x; 1.0251x over previous
"""Trainium2 Bass kernel for BroadcastResidualBlock.

Reference computation (per image, NHWC, H=W=19, C=256, HW=361):
    h1 = relu(bn1(x @ conv1_w + conv1_b))          # 1x1 conv = channel mix
    h2 = relu(dense(h1 over flattened board))       # spatial mix, per channel
    h3 = relu(bn2(h2 @ conv2_w + conv2_b))          # 1x1 conv
    out = x + h3

Strategy: pure data parallel over batch N=256 -> 32 images per core on 8
cores.  BN (inference) folds into the conv weights/biases on the host.  The
host also pre-transposes x into "C-layout" (N, C, HW) so every device-side
matmul contracts over the partition dimension with zero on-device transposes:

    s1: psum[r,  d] += xC_bf16[c_chunk, r_chunk].T @ w1[c_chunk, d]   (h1: S-layout)
    s2: psum[c,  q] += h1[p_chunk, c_chunk].T     @ dw[p_chunk, q]    (h2: C-layout)
    s3: psum[d,  q] += w2[c_chunk, d_chunk].T     @ h2[c_chunk, q]    (h3: C-layout)
    out = relu(psum3) + xC   (single fused VectorE op), stored in C-layout.

Matmuls run in bf16 (fp32 PSUM accumulation); x stays fp32 for the residual.
The host transposes the output back to NHWC.

Schedule: 3-stage software pipeline over images, one 1-bank PSUM tile per
matmul group (7 per image) so releases stagger through the step; epilogues are
split ACT/DVE to balance the engines; x loads ride the sync queue, output
stores the gpsimd (SWDGE) queue so stores never head-of-line-block prefetches.
"""

import numpy as np
import ml_dtypes

import concourse.bass as bass
import concourse.mybir as mybir
import concourse.tile as tile
from concourse import bacc
from concourse.bass_utils import run_bass_kernel_spmd

N_CORES = 8
NIMG = 32            # images per core
C = 256
HW = 361             # 19*19
P = 128
EPS = 1e-3
W_COLS = 2 * C + 3 * HW + 2 * C  # weight blob free size: w1 | dw | w2

F32 = mybir.dt.float32
BF16 = mybir.dt.bfloat16
AF = mybir.ActivationFunctionType
ALU = mybir.AluOpType

# DMA batches: singles at the edges (short critical path at startup/teardown),
# pairs in steady state
BATCHES = [[0], [1]] + [[i, i + 1] for i in range(2, 30, 2)] + [[30], [31]]
BMAX = 2

_prog_cache = {}

BODIES = 1
FUSED_PS = True   # one 2-bank PSUM tile per stage (fewer, larger epilogue ops)
POOL_CFG = dict(xf=6, xb=5, h1=3, h2=4, yo=3)
# per-step emission order of matmul groups: (stage, group)
STEP_ORDER = [(1,0),(2,0),(1,1),(3,0),(1,2),(3,1),(2,1)]


def build_program(has_b1: bool, has_b2: bool, has_b3: bool, reps: int = 1):
    nc = bacc.Bacc("TRN2", target_bir_lowering=False, debug=False)

    xc = nc.dram_tensor("xc", [NIMG, 2, P, HW], F32, kind="ExternalInput").ap()
    wb = nc.dram_tensor("wb", [P, W_COLS], BF16, kind="ExternalInput").ap()
    b1 = b2 = b3 = None
    if has_b1:
        b1 = nc.dram_tensor("b1", [P, 3 * C], F32, kind="ExternalInput").ap()
    if has_b2:
        b2 = nc.dram_tensor("b2", [P, 2, HW], F32, kind="ExternalInput").ap()
    if has_b3:
        b3 = nc.dram_tensor("b3", [2, P], F32, kind="ExternalInput").ap()
    yc = nc.dram_tensor("yc", [NIMG, 2, P, HW], F32, kind="ExternalOutput").ap()

    batch_of = {}
    for bi, imgs in enumerate(BATCHES):
        for k, i in enumerate(imgs):
            batch_of[i] = (bi, k)

    with tile.TileContext(nc) as tc:
        with (
            tc.tile_pool(name="const", bufs=1) as cpool,
            tc.tile_pool(name="xf", bufs=POOL_CFG["xf"]) as xf_pool,
            tc.tile_pool(name="xb", bufs=POOL_CFG["xb"]) as xb_pool,
            tc.tile_pool(name="h1", bufs=POOL_CFG["h1"]) as h1_pool,
            tc.tile_pool(name="h2", bufs=POOL_CFG["h2"]) as h2_pool,
            tc.tile_pool(name="yo", bufs=POOL_CFG["yo"]) as yo_pool,
            tc.tile_pool(name="ps", bufs=(4 if FUSED_PS else 8), space="PSUM") as ps_pool,
        ):
            wsb = cpool.tile([P, W_COLS], BF16)
            # scalar queue: runs in parallel with the first x load on sync;
            # w1 ships first so stage-1 matmuls unblock as early as possible
            nc.scalar.dma_start(wsb[:, : 2 * C], wb[:, : 2 * C])
            nc.scalar.dma_start(wsb[:, 2 * C :], wb[:, 2 * C :])
            O_DW = 2 * C
            O_W2 = 2 * C + 3 * HW

            def w1_ap(cc):                      # [128, 256] rhs for s1
                return wsb[:, cc * C : (cc + 1) * C]

            def dw_ap(pc, k):                   # [k, 361] rhs for s2
                return wsb[:k, O_DW + pc * HW : O_DW + (pc + 1) * HW]

            def w2_ap(cc, dc):                  # [128, 128] lhsT for s3
                o = O_W2 + cc * C + dc * P
                return wsb[:, o : o + P]

            b1sb = b2sb = b3sb = None
            if has_b1:
                b1sb = cpool.tile([P, 3 * C], F32)
                nc.sync.dma_start(b1sb[:], b1)
            if has_b2:
                b2sb = cpool.tile([P, 2, HW], F32)
                nc.sync.dma_start(b2sb[:], b2)
            if has_b3:
                b3sb = cpool.tile([P, 2], F32)
                nc.sync.dma_start(b3sb[:], b3.rearrange("co ci -> ci co"))

            def emit_load0_split():
                # image 0 arrives as two small q-chunks so the very first
                # stage-1 matmul group unblocks as early as possible
                xf0a = cpool.tile([P, 2, 128], F32)
                nc.sync.dma_start(
                    xf0a[:], xc[0][:, :, 0:128].rearrange("co ci q -> ci co q"))
                xb0a = cpool.tile([P, 2, 128], BF16)
                nc.vector.tensor_copy(xb0a[:], xf0a[:])
                xf0b = cpool.tile([P, 2, 233], F32)
                nc.sync.dma_start(
                    xf0b[:], xc[0][:, :, 128:361].rearrange("co ci q -> ci co q"))
                xb0b = cpool.tile([P, 2, 233], BF16)
                nc.vector.tensor_copy(xb0b[:], xf0b[:])

                def s1_lhsT(cc, lo, m):
                    if lo == 0:
                        return xb0a[:, cc, :m]
                    if lo == 128:
                        return xb0b[:, cc, :m]
                    return xb0b[:, cc, 128 : 128 + m]

                resid = lambda dc: [(0, 128, xf0a[:, dc, :]),
                                    (128, 361, xf0b[:, dc, :])]
                return s1_lhsT, (resid, None)

            def emit_load(bi):
                imgs = BATCHES[bi]
                nb = len(imgs)
                xf = xf_pool.tile([P, BMAX, 2, HW], F32, tag="xf", name="xf")
                nc.sync.dma_start(
                    xf[:, :nb],
                    xc[imgs[0] : imgs[0] + nb].rearrange("n co ci q -> ci n co q"))
                xb = xb_pool.tile([P, BMAX, 2, HW], BF16, tag="xb", name="xb")
                nc.vector.tensor_copy(xb[:, :nb], xf[:, :nb])
                return xf, xb

            def emit_s1_group(i, s1_lhsT, rc, h1, pss):
                m = 128 if rc < 2 else 105
                if FUSED_PS:
                    ps = pss["s1"]
                    out = ps[:m, rc * C : rc * C + C]
                else:
                    ps = ps_pool.tile([P, 512], F32, tag="ps", name="ps")
                    out = ps[:m, :C]
                for cc in range(2):
                    nc.tensor.matmul(
                        out,
                        s1_lhsT(cc, rc * 128, m),
                        w1_ap(cc),
                        start=(cc == 0),
                        stop=(cc == 1),
                    )
                if FUSED_PS:
                    if rc < 2:
                        return
                    # single fused epilogue over all three rc slices
                    if b1sb is not None:
                        nc.vector.scalar_tensor_tensor(
                            ps[:, : 3 * C], ps[:, : 3 * C], 0.0, b1sb[:],
                            ALU.bypass, ALU.add)
                    nc.scalar.activation(
                        h1[:].rearrange("p a b -> p (a b)"), ps[:, : 3 * C],
                        AF.Relu)
                    return
                if b1sb is not None:
                    nc.vector.scalar_tensor_tensor(
                        out, out, 0.0,
                        b1sb[:m, rc * C : (rc + 1) * C], ALU.bypass, ALU.add)
                if rc < 2:
                    nc.scalar.activation(h1[:m, rc, :], out, AF.Relu)
                else:
                    nc.vector.tensor_scalar_max(h1[:m, rc, :], out, 0.0)

            def emit_s2_group(i, h1, cc, h2, pss):
                if FUSED_PS:
                    ps = pss["s2"]
                    out = ps[:, cc * 512 : cc * 512 + HW]
                else:
                    ps = ps_pool.tile([P, 512], F32, tag="ps", name="ps")
                    out = ps[:, :HW]
                for pc in range(3):
                    k = 128 if pc < 2 else 105
                    nc.tensor.matmul(
                        out,
                        h1[:k, pc, cc * 128 : (cc + 1) * 128],
                        dw_ap(pc, k),
                        start=(pc == 0),
                        stop=(pc == 2),
                    )
                if FUSED_PS:
                    if cc == 0:
                        return
                    psv = ps.rearrange("p (c x) -> p c x", c=2)[:, :, :HW]
                    if b2sb is not None:
                        nc.vector.scalar_tensor_tensor(
                            psv, psv, 0.0, b2sb[:], ALU.bypass, ALU.add)
                    nc.scalar.activation(h2[:], psv, AF.Relu)
                    return
                if b2sb is not None:
                    nc.vector.scalar_tensor_tensor(
                        out, out, 0.0, b2sb[:, cc, :],
                        ALU.bypass, ALU.add)
                nc.scalar.activation(h2[:, cc, :], out, AF.Relu)

            def emit_s3_group(i, resid_full, k, yo, h2, dc, pss):
                resid, full = resid_full
                if FUSED_PS:
                    ps = pss["s3"]
                    out = ps[:, dc * 512 : dc * 512 + HW]
                else:
                    ps = ps_pool.tile([P, 512], F32, tag="ps", name="ps")
                    out = ps[:, :HW]
                for cc in range(2):
                    nc.tensor.matmul(
                        out,
                        w2_ap(cc, dc),
                        h2[:, cc, :],
                        start=(cc == 0),
                        stop=(cc == 1),
                    )
                if FUSED_PS and full is not None:
                    if dc == 0:
                        return
                    psv = ps.rearrange("p (c x) -> p c x", c=2)[:, :, :HW]
                    xap = full
                    if b3sb is not None:
                        for d2 in range(2):
                            nc.scalar.activation(
                                yo[:, k, d2, :], psv[:, d2, :], AF.Relu,
                                bias=b3sb[:, d2 : d2 + 1])
                        nc.vector.tensor_add(
                            yo[:, k, :, :], yo[:, k, :, :], xap)
                    else:
                        nc.vector.scalar_tensor_tensor(
                            yo[:, k, :, :], psv, 0.0, xap,
                            ALU.max, ALU.add)
                    if i >= NIMG - 2:
                        nc.sync.dma_start(
                            yc[i].rearrange("co ci q -> ci co q"), yo[:, k])
                    return
                if b3sb is not None:
                    nc.scalar.activation(
                        yo[:, k, dc, :], out, AF.Relu,
                        bias=b3sb[:, dc : dc + 1])
                    for qlo, qhi, xap in resid(dc):
                        nc.vector.tensor_add(
                            yo[:, k, dc, qlo:qhi], yo[:, k, dc, qlo:qhi], xap)
                else:
                    for qlo, qhi, xap in resid(dc):
                        nc.vector.scalar_tensor_tensor(
                            yo[:, k, dc, qlo:qhi], out[:, qlo:qhi], 0.0, xap,
                            ALU.max, ALU.add)
                if i >= NIMG - 2:
                    # tail: per-dc store on the (now idle) sync queue starts
                    # draining before the other half's matmuls finish
                    nc.sync.dma_start(yc[i, dc], yo[:, k, dc, :])

            def emit_store(bi, yo):
                imgs = BATCHES[bi]
                nb = len(imgs)
                # SWDGE path: keeps store DMAs (which wait on compute) off the
                # sync queue so they never head-of-line-block prefetch loads
                nc.gpsimd.dma_start(
                    yc[imgs[0] : imgs[0] + nb].rearrange("n co ci q -> ci n co q"),
                    yo[:, :nb])

            def body():
                # software pipeline: s1(i) | s2(i-1) | s3(i-2), interleaved at
                # matmul-group granularity so PSUM slot releases stagger
                s1f, s3f, h1s, h2s, yos = {}, {}, {}, {}, {}

                def load_batch(bi):
                    if bi == 0:
                        s1_lhsT, resid = emit_load0_split()
                        for i in BATCHES[0]:
                            s1f[i], s3f[i] = s1_lhsT, resid
                    else:
                        xf, xb = emit_load(bi)
                        for k, i in enumerate(BATCHES[bi]):
                            s1f[i] = (lambda xb, k: lambda cc, lo, m:
                                      xb[:, k, cc, lo : lo + m])(xb, k)
                            s3f[i] = (
                                (lambda xf, k: lambda dc:
                                 [(0, HW, xf[:, k, dc, :])])(xf, k),
                                xf[:, k, :, :])

                loaded = 0
                for pb in range(4):
                    load_batch(pb)
                    loaded += 1
                for step in range(NIMG + 2):
                    if step % 2 == 0 and loaded < len(BATCHES):
                        load_batch(loaded)
                        loaded += 1
                    i1 = step if step < NIMG else None
                    i2 = step - 1 if 1 <= step <= NIMG else None
                    i3 = step - 2 if step >= 2 else None
                    pss = {}
                    if FUSED_PS:
                        if i1 is not None:
                            pss["s1"] = ps_pool.tile([P, 1024], F32, tag="ps", name="ps1")
                        if i3 is not None:
                            pss["s3"] = ps_pool.tile([P, 1024], F32, tag="ps", name="ps3")
                        if i2 is not None:
                            pss["s2"] = ps_pool.tile([P, 1024], F32, tag="ps", name="ps2")
                    if i1 is not None:
                        h1s[i1] = h1_pool.tile([P, 3, C], BF16, tag="h1", name="h1")
                    bi3 = k3 = None
                    if i3 is not None:
                        bi3, k3 = batch_of[i3]
                        if k3 == 0:
                            yos[bi3] = yo_pool.tile(
                                [P, BMAX, 2, HW], F32, tag="yo", name="yo")
                    if i2 is not None:
                        h2s[i2] = h2_pool.tile([P, 2, HW], BF16, tag="h2", name="h2")
                    # interleave matmul groups per STEP_ORDER
                    for stg, g in STEP_ORDER:
                        if stg == 1 and i1 is not None:
                            emit_s1_group(i1, s1f[i1], g, h1s[i1], pss)
                        elif stg == 3 and i3 is not None:
                            emit_s3_group(i3, s3f[i3], k3, yos[bi3], h2s[i3], g, pss)
                        elif stg == 2 and i2 is not None:
                            emit_s2_group(i2, h1s[i2], g, h2s[i2], pss)
                    if i2 is not None:
                        h1s.pop(i2)
                    if i3 is not None:
                        h2s.pop(i3)
                        s1f.pop(i3, None)
                        s3f.pop(i3, None)
                        if k3 == len(BATCHES[bi3]) - 1 and i3 < NIMG - 2:
                            emit_store(bi3, yos.pop(bi3))

            if reps == 1:
                body()
            else:
                with tc.For_i(0, reps, 1):
                    for _ in range(BODIES):
                        body()

    nc.compile()
    return nc


def _get_program(key):
    if key not in _prog_cache:
        _prog_cache[key] = build_program(*key)
    return _prog_cache[key]


def _marshal(x, conv1_w, conv1_b, bn1_mean, bn1_var, bn1_beta,
             dense_w, dense_b, conv2_w, conv2_b, bn2_mean, bn2_var, bn2_beta):
    bf16 = ml_dtypes.bfloat16
    n = x.shape[0]
    rs1 = 1.0 / np.sqrt(bn1_var.astype(np.float64) + EPS)
    rs2 = 1.0 / np.sqrt(bn2_var.astype(np.float64) + EPS)
    w1f = conv1_w.astype(np.float64) * rs1[None, :]
    w2f = conv2_w.astype(np.float64) * rs2[None, :]
    b1f = (conv1_b - bn1_mean).astype(np.float64) * rs1 + bn1_beta
    b2f = dense_b.astype(np.float64)
    b3f = (conv2_b - bn2_mean).astype(np.float64) * rs2 + bn2_beta
    has_b1 = bool(np.any(b1f != 0.0))
    has_b2 = bool(np.any(b2f != 0.0))
    has_b3 = bool(np.any(b3f != 0.0))

    # weight blob [128, W_COLS]: per partition ci the columns are
    #   w1[cc=0..1] (256 each) | dw[pc=0..2] (361 each) | w2[cc=0..1] (256 each)
    blob = np.zeros((P, W_COLS), np.float64)
    w1r = w1f.reshape(2, P, C)
    for cc in range(2):
        blob[:, cc * C : (cc + 1) * C] = w1r[cc]
    dwp = np.zeros((3 * P, HW), np.float64)
    dwp[:HW] = dense_w
    dwr = dwp.reshape(3, P, HW)
    for pc in range(3):
        blob[:, 2 * C + pc * HW : 2 * C + (pc + 1) * HW] = dwr[pc]
    w2r = w2f.reshape(2, P, C)
    for cc in range(2):
        blob[:, 2 * C + 3 * HW + cc * C : 2 * C + 3 * HW + (cc + 1) * C] = w2r[cc]
    wbb = blob.astype(bf16)

    x_c = np.ascontiguousarray(
        x.reshape(n, HW, C).transpose(0, 2, 1)
    ).reshape(N_CORES, NIMG, 2, P, HW)

    in_maps = []
    for c in range(N_CORES):
        m = {"xc": x_c[c], "wb": wbb}
        if has_b1:
            m["b1"] = np.ascontiguousarray(np.broadcast_to(
                np.tile(b1f, 3).astype(np.float32), (P, 3 * C)))
        if has_b2:
            m["b2"] = np.ascontiguousarray(np.broadcast_to(
                b2f.astype(np.float32), (P, 2, HW)))
        if has_b3:
            m["b3"] = np.ascontiguousarray(
                b3f.astype(np.float32).reshape(2, P))
        in_maps.append(m)
    return (has_b1, has_b2, has_b3), in_maps


def _unmarshal(results, n, h, w):
    y = np.stack([results[c]["yc"] for c in range(N_CORES)])
    y = y.reshape(n, C, HW).transpose(0, 2, 1)
    return np.ascontiguousarray(y.reshape(n, h, w, C).astype(np.float32))


def kernel(x, conv1_w, conv1_b, bn1_mean, bn1_var, bn1_beta,
           dense_w, dense_b, conv2_w, conv2_b, bn2_mean, bn2_var, bn2_beta):
    n, h, w, _ = x.shape
    flags, in_maps = _marshal(
        x, conv1_w, conv1_b, bn1_mean, bn1_var, bn1_beta,
        dense_w, dense_b, conv2_w, conv2_b, bn2_mean, bn2_var, bn2_beta)
    nc = _get_program((*flags, 1))
    res = run_bass_kernel_spmd(nc, in_maps, list(range(N_CORES)))
    return _unmarshal(res.results, n, h, w)


# revision 18
# speedup vs baseline: 1.4259x; 1.0257x over previous
"""Trainium2 Bass kernel for BroadcastResidualBlock.

Reference computation (per image, NHWC, H=W=19, C=256, HW=361):
    h1 = relu(bn1(x @ conv1_w + conv1_b))          # 1x1 conv = channel mix
    h2 = relu(dense(h1 over flattened board))       # spatial mix, per channel
    h3 = relu(bn2(h2 @ conv2_w + conv2_b))          # 1x1 conv
    out = x + h3

Strategy: pure data parallel over batch N=256 -> 32 images per core on 8
cores.  BN (inference) folds into the conv weights/biases on the host.  The
host also pre-transposes x into "C-layout" (N, C, HW) so every device-side
matmul contracts over the partition dimension with zero on-device transposes:

    s1: psum[r,  d] += xC_bf16[c_chunk, r_chunk].T @ w1[c_chunk, d]   (h1: S-layout)
    s2: psum[c,  q] += h1[p_chunk, c_chunk].T     @ dw[p_chunk, q]    (h2: C-layout)
    s3: psum[d,  q] += w2[c_chunk, d_chunk].T     @ h2[c_chunk, q]    (h3: C-layout)
    out = relu(psum3) + xC   (single fused VectorE op), stored in C-layout.

Matmuls run in bf16 (fp32 PSUM accumulation); x stays fp32 for the residual.
The host transposes the output back to NHWC.

Schedule: 3-stage software pipeline over images, one 1-bank PSUM tile per
matmul group (7 per image) so releases stagger through the step; epilogues are
split ACT/DVE to balance the engines; x loads ride the sync queue, output
stores the gpsimd (SWDGE) queue so stores never head-of-line-block prefetches.
"""

import numpy as np
import ml_dtypes

import concourse.bass as bass
import concourse.mybir as mybir
import concourse.tile as tile
from concourse import bacc
from concourse.bass_utils import run_bass_kernel_spmd

N_CORES = 8
NIMG = 32            # images per core
C = 256
HW = 361             # 19*19
P = 128
EPS = 1e-3
W_COLS = 2 * C + 3 * HW + 2 * C  # weight blob free size: w1 | dw | w2

F32 = mybir.dt.float32
BF16 = mybir.dt.bfloat16
AF = mybir.ActivationFunctionType
ALU = mybir.AluOpType

# DMA batches: singles at the edges (short critical path at startup/teardown),
# pairs in steady state
BATCHES = [[0], [1]] + [[i, i + 1] for i in range(2, 30, 2)] + [[30], [31]]
BMAX = 2

_prog_cache = {}

BODIES = 1
FUSED_PS = True   # one 2-bank PSUM tile per stage (fewer, larger epilogue ops)
POOL_CFG = dict(xf=6, xb=5, h1=3, h2=4, yo=3)
# per-step emission order of matmul groups: (stage, group)
STEP_ORDER = [(1,0),(2,0),(1,1),(3,0),(1,2),(3,1),(2,1)]


def build_program(has_b1: bool, has_b2: bool, has_b3: bool, reps: int = 1):
    nc = bacc.Bacc("TRN2", target_bir_lowering=False, debug=False)

    xc = nc.dram_tensor("xc", [NIMG, 2, P, HW], BF16, kind="ExternalInput").ap()
    wb = nc.dram_tensor("wb", [P, W_COLS], BF16, kind="ExternalInput").ap()
    b1 = b2 = b3 = None
    if has_b1:
        b1 = nc.dram_tensor("b1", [P, 3 * C], F32, kind="ExternalInput").ap()
    if has_b2:
        b2 = nc.dram_tensor("b2", [P, 2, HW], F32, kind="ExternalInput").ap()
    if has_b3:
        b3 = nc.dram_tensor("b3", [2, P], F32, kind="ExternalInput").ap()
    yc = nc.dram_tensor("yc", [NIMG, 2, P, HW], F32, kind="ExternalOutput").ap()

    batch_of = {}
    for bi, imgs in enumerate(BATCHES):
        for k, i in enumerate(imgs):
            batch_of[i] = (bi, k)

    with tile.TileContext(nc) as tc:
        with (
            tc.tile_pool(name="const", bufs=1) as cpool,
            tc.tile_pool(name="xf", bufs=POOL_CFG["xf"]) as xf_pool,
            tc.tile_pool(name="h1", bufs=POOL_CFG["h1"]) as h1_pool,
            tc.tile_pool(name="h2", bufs=POOL_CFG["h2"]) as h2_pool,
            tc.tile_pool(name="yo", bufs=POOL_CFG["yo"]) as yo_pool,
            tc.tile_pool(name="ps", bufs=(4 if FUSED_PS else 8), space="PSUM") as ps_pool,
        ):
            wsb = cpool.tile([P, W_COLS], BF16)
            # scalar queue: runs in parallel with the first x load on sync;
            # w1 ships first so stage-1 matmuls unblock as early as possible
            nc.scalar.dma_start(wsb[:, : 2 * C], wb[:, : 2 * C])
            nc.scalar.dma_start(wsb[:, 2 * C :], wb[:, 2 * C :])
            O_DW = 2 * C
            O_W2 = 2 * C + 3 * HW

            def w1_ap(cc):                      # [128, 256] rhs for s1
                return wsb[:, cc * C : (cc + 1) * C]

            def dw_ap(pc, k):                   # [k, 361] rhs for s2
                return wsb[:k, O_DW + pc * HW : O_DW + (pc + 1) * HW]

            def w2_ap(cc, dc):                  # [128, 128] lhsT for s3
                o = O_W2 + cc * C + dc * P
                return wsb[:, o : o + P]

            b1sb = b2sb = b3sb = None
            if has_b1:
                b1sb = cpool.tile([P, 3 * C], F32)
                nc.sync.dma_start(b1sb[:], b1)
            if has_b2:
                b2sb = cpool.tile([P, 2, HW], F32)
                nc.sync.dma_start(b2sb[:], b2)
            if has_b3:
                b3sb = cpool.tile([P, 2], F32)
                nc.sync.dma_start(b3sb[:], b3.rearrange("co ci -> ci co"))

            def emit_load0_split():
                # image 0 arrives as two small q-chunks so the very first
                # stage-1 matmul group unblocks as early as possible
                xb0a = cpool.tile([P, 2, 128], BF16)
                nc.sync.dma_start(
                    xb0a[:], xc[0][:, :, 0:128].rearrange("co ci q -> ci co q"))
                xb0b = cpool.tile([P, 2, 233], BF16)
                nc.sync.dma_start(
                    xb0b[:], xc[0][:, :, 128:361].rearrange("co ci q -> ci co q"))

                def s1_lhsT(cc, lo, m):
                    if lo == 0:
                        return xb0a[:, cc, :m]
                    if lo == 128:
                        return xb0b[:, cc, :m]
                    return xb0b[:, cc, 128 : 128 + m]

                resid = lambda dc: [(0, 128, xb0a[:, dc, :]),
                                    (128, 361, xb0b[:, dc, :])]
                return s1_lhsT, (resid, None)

            def emit_load(bi):
                imgs = BATCHES[bi]
                nb = len(imgs)
                xf = xf_pool.tile([P, BMAX, 2, HW], BF16, tag="xf", name="xf")
                nc.sync.dma_start(
                    xf[:, :nb],
                    xc[imgs[0] : imgs[0] + nb].rearrange("n co ci q -> ci n co q"))
                return xf

            def emit_s1_group(i, s1_lhsT, rc, h1, pss):
                m = 128 if rc < 2 else 105
                if FUSED_PS:
                    ps = pss["s1"]
                    out = ps[:m, rc * C : rc * C + C]
                else:
                    ps = ps_pool.tile([P, 512], F32, tag="ps", name="ps")
                    out = ps[:m, :C]
                for cc in range(2):
                    nc.tensor.matmul(
                        out,
                        s1_lhsT(cc, rc * 128, m),
                        w1_ap(cc),
                        start=(cc == 0),
                        stop=(cc == 1),
                    )
                if FUSED_PS:
                    if rc < 2:
                        return
                    # single fused epilogue over all three rc slices
                    if b1sb is not None:
                        nc.vector.scalar_tensor_tensor(
                            ps[:, : 3 * C], ps[:, : 3 * C], 0.0, b1sb[:],
                            ALU.bypass, ALU.add)
                    nc.scalar.activation(
                        h1[:].rearrange("p a b -> p (a b)"), ps[:, : 3 * C],
                        AF.Relu)
                    return
                if b1sb is not None:
                    nc.vector.scalar_tensor_tensor(
                        out, out, 0.0,
                        b1sb[:m, rc * C : (rc + 1) * C], ALU.bypass, ALU.add)
                if rc < 2:
                    nc.scalar.activation(h1[:m, rc, :], out, AF.Relu)
                else:
                    nc.vector.tensor_scalar_max(h1[:m, rc, :], out, 0.0)

            def emit_s2_group(i, h1, cc, h2, pss):
                if FUSED_PS:
                    ps = pss["s2"]
                    out = ps[:, cc * 512 : cc * 512 + HW]
                else:
                    ps = ps_pool.tile([P, 512], F32, tag="ps", name="ps")
                    out = ps[:, :HW]
                for pc in range(3):
                    k = 128 if pc < 2 else 105
                    nc.tensor.matmul(
                        out,
                        h1[:k, pc, cc * 128 : (cc + 1) * 128],
                        dw_ap(pc, k),
                        start=(pc == 0),
                        stop=(pc == 2),
                    )
                if FUSED_PS:
                    if cc == 0:
                        return
                    psv = ps.rearrange("p (c x) -> p c x", c=2)[:, :, :HW]
                    if b2sb is not None:
                        nc.vector.scalar_tensor_tensor(
                            psv, psv, 0.0, b2sb[:], ALU.bypass, ALU.add)
                    nc.scalar.activation(h2[:], psv, AF.Relu)
                    return
                if b2sb is not None:
                    nc.vector.scalar_tensor_tensor(
                        out, out, 0.0, b2sb[:, cc, :],
                        ALU.bypass, ALU.add)
                nc.scalar.activation(h2[:, cc, :], out, AF.Relu)

            def emit_s3_group(i, resid_full, k, yo, h2, dc, pss):
                resid, full = resid_full
                if FUSED_PS:
                    ps = pss["s3"]
                    out = ps[:, dc * 512 : dc * 512 + HW]
                else:
                    ps = ps_pool.tile([P, 512], F32, tag="ps", name="ps")
                    out = ps[:, :HW]
                for cc in range(2):
                    nc.tensor.matmul(
                        out,
                        w2_ap(cc, dc),
                        h2[:, cc, :],
                        start=(cc == 0),
                        stop=(cc == 1),
                    )
                if FUSED_PS and full is not None:
                    if dc == 0:
                        return
                    psv = ps.rearrange("p (c x) -> p c x", c=2)[:, :, :HW]
                    xap = full
                    if b3sb is not None:
                        for d2 in range(2):
                            nc.scalar.activation(
                                yo[:, k, d2, :], psv[:, d2, :], AF.Relu,
                                bias=b3sb[:, d2 : d2 + 1])
                        nc.vector.tensor_add(
                            yo[:, k, :, :], yo[:, k, :, :], xap)
                    else:
                        nc.vector.scalar_tensor_tensor(
                            yo[:, k, :, :], psv, 0.0, xap,
                            ALU.max, ALU.add)
                    if i >= NIMG - 2:
                        nc.sync.dma_start(
                            yc[i].rearrange("co ci q -> ci co q"), yo[:, k])
                    return
                if b3sb is not None:
                    nc.scalar.activation(
                        yo[:, k, dc, :], out, AF.Relu,
                        bias=b3sb[:, dc : dc + 1])
                    for qlo, qhi, xap in resid(dc):
                        nc.vector.tensor_add(
                            yo[:, k, dc, qlo:qhi], yo[:, k, dc, qlo:qhi], xap)
                else:
                    for qlo, qhi, xap in resid(dc):
                        nc.vector.scalar_tensor_tensor(
                            yo[:, k, dc, qlo:qhi], out[:, qlo:qhi], 0.0, xap,
                            ALU.max, ALU.add)
                if i >= NIMG - 2:
                    # tail: per-dc store on the (now idle) sync queue starts
                    # draining before the other half's matmuls finish
                    nc.sync.dma_start(yc[i, dc], yo[:, k, dc, :])

            def emit_store(bi, yo):
                imgs = BATCHES[bi]
                nb = len(imgs)
                # SWDGE path: keeps store DMAs (which wait on compute) off the
                # sync queue so they never head-of-line-block prefetch loads
                nc.gpsimd.dma_start(
                    yc[imgs[0] : imgs[0] + nb].rearrange("n co ci q -> ci n co q"),
                    yo[:, :nb])

            def body():
                # software pipeline: s1(i) | s2(i-1) | s3(i-2), interleaved at
                # matmul-group granularity so PSUM slot releases stagger
                s1f, s3f, h1s, h2s, yos = {}, {}, {}, {}, {}

                def load_batch(bi):
                    if bi == 0:
                        s1_lhsT, resid = emit_load0_split()
                        for i in BATCHES[0]:
                            s1f[i], s3f[i] = s1_lhsT, resid
                    else:
                        xf = emit_load(bi)
                        for k, i in enumerate(BATCHES[bi]):
                            s1f[i] = (lambda xf, k: lambda cc, lo, m:
                                      xf[:, k, cc, lo : lo + m])(xf, k)
                            s3f[i] = (
                                (lambda xf, k: lambda dc:
                                 [(0, HW, xf[:, k, dc, :])])(xf, k),
                                xf[:, k, :, :])

                loaded = 0
                for pb in range(4):
                    load_batch(pb)
                    loaded += 1
                for step in range(NIMG + 2):
                    if step % 2 == 0 and loaded < len(BATCHES):
                        load_batch(loaded)
                        loaded += 1
                    i1 = step if step < NIMG else None
                    i2 = step - 1 if 1 <= step <= NIMG else None
                    i3 = step - 2 if step >= 2 else None
                    pss = {}
                    if FUSED_PS:
                        if i1 is not None:
                            pss["s1"] = ps_pool.tile([P, 1024], F32, tag="ps", name="ps1")
                        if i3 is not None:
                            pss["s3"] = ps_pool.tile([P, 1024], F32, tag="ps", name="ps3")
                        if i2 is not None:
                            pss["s2"] = ps_pool.tile([P, 1024], F32, tag="ps", name="ps2")
                    if i1 is not None:
                        h1s[i1] = h1_pool.tile([P, 3, C], BF16, tag="h1", name="h1")
                    bi3 = k3 = None
                    if i3 is not None:
                        bi3, k3 = batch_of[i3]
                        if k3 == 0:
                            yos[bi3] = yo_pool.tile(
                                [P, BMAX, 2, HW], F32, tag="yo", name="yo")
                    if i2 is not None:
                        h2s[i2] = h2_pool.tile([P, 2, HW], BF16, tag="h2", name="h2")
                    # interleave matmul groups per STEP_ORDER
                    for stg, g in STEP_ORDER:
                        if stg == 1 and i1 is not None:
                            emit_s1_group(i1, s1f[i1], g, h1s[i1], pss)
                        elif stg == 3 and i3 is not None:
                            emit_s3_group(i3, s3f[i3], k3, yos[bi3], h2s[i3], g, pss)
                        elif stg == 2 and i2 is not None:
                            emit_s2_group(i2, h1s[i2], g, h2s[i2], pss)
                    if i2 is not None:
                        h1s.pop(i2)
                    if i3 is not None:
                        h2s.pop(i3)
                        s1f.pop(i3, None)
                        s3f.pop(i3, None)
                        if k3 == len(BATCHES[bi3]) - 1 and i3 < NIMG - 2:
                            emit_store(bi3, yos.pop(bi3))

            if reps == 1:
                body()
            else:
                with tc.For_i(0, reps, 1):
                    for _ in range(BODIES):
                        body()

    nc.compile()
    return nc


def _get_program(key):
    if key not in _prog_cache:
        _prog_cache[key] = build_program(*key)
    return _prog_cache[key]


def _marshal(x, conv1_w, conv1_b, bn1_mean, bn1_var, bn1_beta,
             dense_w, dense_b, conv2_w, conv2_b, bn2_mean, bn2_var, bn2_beta):
    bf16 = ml_dtypes.bfloat16
    n = x.shape[0]
    rs1 = 1.0 / np.sqrt(bn1_var.astype(np.float64) + EPS)
    rs2 = 1.0 / np.sqrt(bn2_var.astype(np.float64) + EPS)
    w1f = conv1_w.astype(np.float64) * rs1[None, :]
    w2f = conv2_w.astype(np.float64) * rs2[None, :]
    b1f = (conv1_b - bn1_mean).astype(np.float64) * rs1 + bn1_beta
    b2f = dense_b.astype(np.float64)
    b3f = (conv2_b - bn2_mean).astype(np.float64) * rs2 + bn2_beta
    has_b1 = bool(np.any(b1f != 0.0))
    has_b2 = bool(np.any(b2f != 0.0))
    has_b3 = bool(np.any(b3f != 0.0))

    # weight blob [128, W_COLS]: per partition ci the columns are
    #   w1[cc=0..1] (256 each) | dw[pc=0..2] (361 each) | w2[cc=0..1] (256 each)
    blob = np.zeros((P, W_COLS), np.float64)
    w1r = w1f.reshape(2, P, C)
    for cc in range(2):
        blob[:, cc * C : (cc + 1) * C] = w1r[cc]
    dwp = np.zeros((3 * P, HW), np.float64)
    dwp[:HW] = dense_w
    dwr = dwp.reshape(3, P, HW)
    for pc in range(3):
        blob[:, 2 * C + pc * HW : 2 * C + (pc + 1) * HW] = dwr[pc]
    w2r = w2f.reshape(2, P, C)
    for cc in range(2):
        blob[:, 2 * C + 3 * HW + cc * C : 2 * C + 3 * HW + (cc + 1) * C] = w2r[cc]
    wbb = blob.astype(bf16)

    x_c = np.ascontiguousarray(
        x.reshape(n, HW, C).transpose(0, 2, 1).astype(bf16)
    ).reshape(N_CORES, NIMG, 2, P, HW)

    in_maps = []
    for c in range(N_CORES):
        m = {"xc": x_c[c], "wb": wbb}
        if has_b1:
            m["b1"] = np.ascontiguousarray(np.broadcast_to(
                np.tile(b1f, 3).astype(np.float32), (P, 3 * C)))
        if has_b2:
            m["b2"] = np.ascontiguousarray(np.broadcast_to(
                b2f.astype(np.float32), (P, 2, HW)))
        if has_b3:
            m["b3"] = np.ascontiguousarray(
                b3f.astype(np.float32).reshape(2, P))
        in_maps.append(m)
    return (has_b1, has_b2, has_b3), in_maps


def _unmarshal(results, n, h, w):
    y = np.stack([results[c]["yc"] for c in range(N_CORES)])
    y = y.reshape(n, C, HW).transpose(0, 2, 1)
    return np.ascontiguousarray(y.reshape(n, h, w, C).astype(np.float32))


def kernel(x, conv1_w, conv1_b, bn1_mean, bn1_var, bn1_beta,
           dense_w, dense_b, conv2_w, conv2_b, bn2_mean, bn2_var, bn2_beta):
    n, h, w, _ = x.shape
    flags, in_maps = _marshal(
        x, conv1_w, conv1_b, bn1_mean, bn1_var, bn1_beta,
        dense_w, dense_b, conv2_w, conv2_b, bn2_mean, bn2_var, bn2_beta)
    nc = _get_program((*flags, 1))
    res = run_bass_kernel_spmd(nc, in_maps, list(range(N_CORES)))
    return _unmarshal(res.results, n, h, w)
